# revision 1
# baseline (speedup 1.0000x reference)
"""Conv2d(128->256, 3x3, pad=1) over (32,128,56,56), data-parallel across 8
NeuronCores (4 images per core).

Per core: conv = 9 shifted accumulating matmuls per output tile.
  - contraction K = Cin = 128 (partition dim)
  - stationary lhsT = W^T[ci, co_tile] per (ky,kx)  -> [128, 128]
  - moving rhs = input pixels [128, 8 rows, 56 cols] (N = 448 <= 512)
  - PSUM accumulates the 9 (ky,kx) contributions; edge handling via
    rectangular sub-range matmuls (center tap first with start=True covers
    the full tile, so partial writes accumulate on top).
Bias is added during the PSUM->SBUF copy (ScalarE/VectorE alternating).
"""

import numpy as np
import ml_dtypes

import concourse.mybir as mybir
import concourse.tile as tile
from concourse import bacc
from concourse.bass_utils import run_bass_kernel_spmd

N_CORES = 8
B, CIN, H, W = 32, 128, 56, 56
COUT, R, S = 256, 3, 3
BL = B // N_CORES          # images per core
NCOT = COUT // 128         # Cout tiles of 128
YCHUNK = 8                 # output rows per matmul tile
NYC = H // YCHUNK

MM_DT = mybir.dt.bfloat16  # matmul operand dtype on device
MM_NP = ml_dtypes.bfloat16  # matching numpy dtype for host-side cast

_cache = {}


def _build():
    if "nc" in _cache:
        return _cache["nc"]
    nc = bacc.Bacc("TRN2", target_bir_lowering=False, debug=False)
    f32 = mybir.dt.float32
    x_d = nc.dram_tensor("x", [BL, CIN, H, W], MM_DT, kind="ExternalInput").ap()
    w_d = nc.dram_tensor("w", [CIN, NCOT, R, S, 128], MM_DT, kind="ExternalInput").ap()
    b_d = nc.dram_tensor("b", [128, NCOT], f32, kind="ExternalInput").ap()
    y_d = nc.dram_tensor("y", [BL, COUT, H, W], f32, kind="ExternalOutput").ap()

    with tile.TileContext(nc) as tc:
        with (
            tc.tile_pool(name="consts", bufs=1) as cpool,
            tc.tile_pool(name="xin", bufs=2) as xpool,
            tc.tile_pool(name="yout", bufs=2) as opool,
            tc.tile_pool(name="ps", bufs=8, space="PSUM") as pspool,
        ):
            w_sb = cpool.tile([CIN, NCOT, R, S, 128], MM_DT)
            nc.sync.dma_start(w_sb[:], w_d[:])
            b_sb = cpool.tile([128, NCOT], f32)
            nc.sync.dma_start(b_sb[:], b_d[:])

            for img in range(BL):
                x_sb = xpool.tile([CIN, H, W], MM_DT, name=f"x_sb_{img}", tag="x_sb")
                nc.sync.dma_start(x_sb[:], x_d[img])
                for cot in range(NCOT):
                    o_sb = opool.tile(
                        [128, H, W], f32, name=f"o_sb_{img}_{cot}", tag="o_sb"
                    )
                    for yc in range(NYC):
                        y0 = YCHUNK * yc
                        ps = pspool.tile(
                            [128, YCHUNK, W], f32, name=f"ps_{img}_{cot}_{yc}", tag="ps"
                        )
                        # center tap first: full-tile write with start=True
                        nc.tensor.matmul(
                            ps[:],
                            w_sb[:, cot, 1, 1, :],
                            x_sb[:, y0 : y0 + YCHUNK, :],
                            start=True,
                            stop=False,
                        )
                        for ky in range(R):
                            for kx in range(S):
                                if ky == 1 and kx == 1:
                                    continue
                                oy0 = max(0, 1 - ky - y0)
                                oy1 = min(YCHUNK, H + 1 - y0 - ky)
                                ox0 = max(0, 1 - kx)
                                ox1 = min(W, W + 1 - kx)
                                iy0 = y0 + oy0 + ky - 1
                                iy1 = y0 + oy1 + ky - 1
                                ix0 = ox0 + kx - 1
                                ix1 = ox1 + kx - 1
                                nc.tensor.matmul(
                                    ps[:, oy0:oy1, ox0:ox1],
                                    w_sb[:, cot, ky, kx, :],
                                    x_sb[:, iy0:iy1, ix0:ix1],
                                    start=False,
                                    stop=(ky == 2 and kx == 2),
                                )
                        # PSUM -> SBUF with fused bias add; alternate engines
                        if yc % 2 == 0:
                            nc.scalar.activation(
                                o_sb[:, y0 : y0 + YCHUNK, :],
                                ps[:],
                                mybir.ActivationFunctionType.Identity,
                                bias=b_sb[:, cot : cot + 1],
                            )
                        else:
                            nc.vector.tensor_scalar_add(
                                o_sb[:, y0 : y0 + YCHUNK, :],
                                ps[:],
                                b_sb[:, cot : cot + 1],
                            )
                    nc.sync.dma_start(y_d[img, 128 * cot : 128 * (cot + 1)], o_sb[:])

    nc.compile()
    _cache["nc"] = nc
    return nc


def kernel(inputs, weight, bias):
    nc = _build()
    np_dt = MM_NP if MM_DT == mybir.dt.bfloat16 else np.float32
    x = np.asarray(inputs).astype(np_dt)
    # weight (co, ci, ky, kx) -> (ci, cot, ky, kx, co_in_tile)
    w = np.ascontiguousarray(
        np.asarray(weight)
        .reshape(NCOT, 128, CIN, R, S)
        .transpose(2, 0, 3, 4, 1)
        .astype(np_dt)
    )
    b = np.ascontiguousarray(
        np.asarray(bias).astype(np.float32).reshape(NCOT, 128).T
    )
    in_maps = [
        {"x": np.ascontiguousarray(x[c * BL : (c + 1) * BL]), "w": w, "b": b}
        for c in range(N_CORES)
    ]
    res = run_bass_kernel_spmd(nc, in_maps, core_ids=list(range(N_CORES)))
    return np.concatenate([res.results[c]["y"] for c in range(N_CORES)], axis=0)
